# revision 3
# baseline (speedup 1.0000x reference)
"""Causal multi-head attention (B=2, S=2048, D=1024, 16 heads of 64) on 8 TRN2
NeuronCores.

Sharding: core c -> batch b = c//4, head-group g = c%4 (4 heads = 256 model
dims per core).  Wq/Wk/Wv column-parallel, Wo row-parallel; the 4 partial
outputs per batch are summed on the host (no collectives).

Per-core data flow (all matmul compute in bf16, fp32 PSUM accumulation):
  QT = (Wq_g/8) @ x^T      [256, 2048]   (scale 1/sqrt(hd) folded into Wq,bq)
  KT = Wk_g @ x^T          [256, 2048]
  V  = x @ Wv_g^T + bv     [2048, 256]   natural layout, ones-augmented
  per head h, per 512-wide query block j:
    ST[sk,sq] = K_h @ Q_h^T   (two heads packed via 64-row PE tiling)
    P = exp(ST + causal mask)               (ACT, fp32 -> bf16)
    preoutT_aug[65, sq] += V_aug_h^T-style matmul over sk chunks
      rows 0..63 = V_h^T @ P (unnormalized), row 64 = column sums l[sq]
    preoutT = preoutT_aug[0:64] * (1/l)     (softmax normalization)
  out_partial = preoutT.T @ Wo_g^T          [2048, 1024] fp32
Host: out[b] = sum of the 4 head-group partials + bo.
"""

import numpy as np
import ml_dtypes

B, S, D = 2, 2048, 1024
HD = 64
NH = D // HD
N_CORES = 8
GROUPS = 4          # head-groups (tensor-parallel)
JG = D // GROUPS    # local dims per core = 256
NHL = JG // HD      # local heads = 4
KCH = D // 128      # contraction chunks for projections = 8
NKT = S // 128      # sk tiles = 16
NJB = S // 512      # query blocks of 512 = 4
MASK_VAL = -1e9

BF16 = ml_dtypes.bfloat16

_cached = {}


def _build():
    import concourse.bacc as bacc
    import concourse.tile as tile
    import concourse.mybir as mybir

    f32 = mybir.dt.float32
    bf16 = mybir.dt.bfloat16
    Ident = mybir.ActivationFunctionType.Identity
    Exp = mybir.ActivationFunctionType.Exp

    nc = bacc.Bacc("TRN2", target_bir_lowering=False, debug=False,
                   num_devices=N_CORES)

    xT = nc.dram_tensor("xT", [D, S], bf16, kind="ExternalInput").ap()
    wqT = nc.dram_tensor("wqT", [D, JG], bf16, kind="ExternalInput").ap()
    wkT = nc.dram_tensor("wkT", [D, JG], bf16, kind="ExternalInput").ap()
    wvT = nc.dram_tensor("wvT", [D, JG], bf16, kind="ExternalInput").ap()
    woT = nc.dram_tensor("woT", [JG, D], bf16, kind="ExternalInput").ap()
    bqc = nc.dram_tensor("bqc", [JG, 1], f32, kind="ExternalInput").ap()
    bkc = nc.dram_tensor("bkc", [JG, 1], f32, kind="ExternalInput").ap()
    bvb = nc.dram_tensor("bvb", [128, JG], f32, kind="ExternalInput").ap()
    maskT = nc.dram_tensor("maskT", [128, 128], f32, kind="ExternalInput").ap()
    out = nc.dram_tensor("out", [S, D], f32, kind="ExternalOutput").ap()

    with tile.TileContext(nc) as tc:
        with (
            tc.tile_pool(name="const", bufs=1) as cpool,
            tc.tile_pool(name="qkt", bufs=1) as qkt_pool,
            tc.tile_pool(name="pbig", bufs=3) as p_pool,
            tc.tile_pool(name="small", bufs=4) as small_pool,
            tc.tile_pool(name="outp", bufs=4) as out_pool,
            tc.tile_pool(name="mm_ps", bufs=4, space="PSUM") as mm_ps,
            tc.tile_pool(name="po_ps", bufs=4, space="PSUM") as po_ps,
        ):
            # ---- constants / weights in SBUF ----
            xt_all = cpool.tile([128, KCH, S], bf16)
            nc.sync.dma_start(
                xt_all[:], xT.rearrange("(k p) s -> p k s", p=128))
            wq_sb = cpool.tile([128, KCH, JG], bf16)
            nc.sync.dma_start(
                wq_sb[:], wqT.rearrange("(k p) m -> p k m", p=128))
            wk_sb = cpool.tile([128, KCH, JG], bf16)
            nc.sync.dma_start(
                wk_sb[:], wkT.rearrange("(k p) m -> p k m", p=128))
            wv_sb = cpool.tile([128, KCH, JG], bf16)
            nc.sync.dma_start(
                wv_sb[:], wvT.rearrange("(k p) m -> p k m", p=128))
            wo_sb = cpool.tile([128, 2, D], bf16)
            nc.sync.dma_start(
                wo_sb[:], woT.rearrange("(t p) n -> p t n", p=128))
            bq_sb = cpool.tile([128, 2], f32)
            nc.sync.dma_start(bq_sb[:], bqc.rearrange("(t p) o -> p (t o)", p=128))
            bk_sb = cpool.tile([128, 2], f32)
            nc.sync.dma_start(bk_sb[:], bkc.rearrange("(t p) o -> p (t o)", p=128))
            bvb_sb = cpool.tile([128, JG], f32)
            nc.sync.dma_start(bvb_sb[:], bvb[:])
            mask_sb = cpool.tile([128, 128], f32)
            nc.sync.dma_start(mask_sb[:], maskT[:])

            qt = [cpool.tile([128, S], bf16, name=f"qt{t}") for t in range(2)]
            kt = [cpool.tile([128, S], bf16, name=f"kt{t}") for t in range(2)]
            # V, ones-augmented: per sk-chunk ki and local head h the AV
            # stationary operand is v_all[:, ki, 65h : 65h+65]
            v_all = cpool.tile([128, NKT, NHL * 65], bf16)
            nc.vector.memset(
                v_all.rearrange("p k (h c) -> p k h c", c=65)[:, :, :, 64:65], 1.0)
            # normalized attention output, transposed: pair t holds heads
            # 2t (partitions 0-63) and 2t+1 (partitions 64-127)
            po = [cpool.tile([128, S], bf16, name=f"po{t}") for t in range(2)]

            # ---- Q^T / K^T projections ----
            for w_sb, b_sb, dst in ((wq_sb, bq_sb, qt), (wk_sb, bk_sb, kt)):
                for t in range(2):
                    ps = [mm_ps.tile([128, 512], f32, tag="mm", name=f"psproj{t}_{n}")
                          for n in range(4)]
                    for k in range(KCH):
                        lhsT = w_sb[:, k, 128 * t:128 * t + 128]
                        for n in range(4):
                            nc.tensor.matmul(
                                ps[n][:],
                                lhsT=lhsT,
                                rhs=xt_all[:, k, 512 * n:512 * n + 512],
                                start=(k == 0), stop=(k == KCH - 1))
                    for n in range(4):
                        nc.scalar.activation(
                            dst[t][:, 512 * n:512 * n + 512], ps[n][:],
                            Ident, bias=b_sb[:, t:t + 1])

            # ---- V projection (natural layout) ----
            for si in range(NKT):
                ps = mm_ps.tile([128, JG], f32, tag="mm")
                for k in range(KCH):
                    nc.tensor.matmul(
                        ps[:],
                        lhsT=xt_all[:, k, 128 * si:128 * si + 128],
                        rhs=wv_sb[:, k, :],
                        start=(k == 0), stop=(k == KCH - 1))
                nc.vector.tensor_add(
                    v_all[:, si, :].rearrange("p (h c) -> p h c", c=65)[:, :, 0:64],
                    ps.rearrange("p (h c) -> p h c", c=64),
                    bvb_sb.rearrange("p (h c) -> p h c", c=64))

            # ---- attention: heads packed in pairs via 64-row PE tiling ----
            for pair in range(2):
                qt_t, kt_t = qt[pair], kt[pair]
                for j in range(NJB):
                    nk = 4 * (j + 1)
                    pt = [p_pool.tile([128, NKT, 512], bf16, tag="p",
                                      name=f"pt{pair}_{j}_{hh}")
                          for hh in range(2)]
                    pos = [po_ps.tile([65, 512], f32, tag="po",
                                      name=f"pos{pair}_{j}_{hh}")
                           for hh in range(2)]
                    # scores + exp
                    for ki in range(nk):
                        d = max(0, 128 * ki - 512 * j)
                        for hh in range(2):
                            base = 64 * hh
                            st = mm_ps.tile([128, 512], f32, tag="mm")
                            nc.tensor.matmul(
                                st[:, d:512],
                                lhsT=kt_t[base:base + 64,
                                          128 * ki:128 * ki + 128],
                                rhs=qt_t[base:base + 64,
                                         512 * j + d:512 * j + 512],
                                start=True, stop=True)
                            if ki >= 4 * j:
                                nc.vector.tensor_add(
                                    st[:, d:d + 128], st[:, d:d + 128],
                                    mask_sb[:])
                            nc.scalar.activation(
                                pt[hh][:, ki, d:512], st[:, d:512], Exp)
                    # A @ V accumulation (rows 0..63) + row sums (row 64)
                    for ki in range(nk):
                        d = max(0, 128 * ki - 512 * j)
                        for hh in range(2):
                            h = 2 * pair + hh
                            nc.tensor.matmul(
                                pos[hh][0:65, d:512],
                                lhsT=v_all[:, ki, 65 * h:65 * h + 65],
                                rhs=pt[hh][:, ki, d:512],
                                start=(ki == 0), stop=(ki == nk - 1))
                    # normalize: preoutT = preoutT_unnorm * (1/l)
                    for hh in range(2):
                        recip = small_pool.tile([1, 512], f32, tag="recip")
                        nc.vector.reciprocal(recip[:], pos[hh][64:65, 0:512])
                        rb = small_pool.tile([64, 512], f32, tag="rb")
                        nc.gpsimd.partition_broadcast(rb[:], recip[:])
                        nc.vector.tensor_mul(
                            po[pair][64 * hh:64 * hh + 64,
                                     512 * j:512 * j + 512],
                            pos[hh][0:64, 0:512], rb[:])

            # ---- output projection ----
            for m in range(NKT):
                for n in range(2):
                    ps = mm_ps.tile([128, 512], f32, tag="mm")
                    for t in range(2):
                        nc.tensor.matmul(
                            ps[:],
                            lhsT=po[t][:, 128 * m:128 * m + 128],
                            rhs=wo_sb[:, t, 512 * n:512 * n + 512],
                            start=(t == 0), stop=(t == 1))
                    ob = out_pool.tile([128, 512], f32, tag="ob")
                    nc.vector.tensor_copy(ob[:], ps[:])
                    nc.sync.dma_start(
                        out[128 * m:128 * m + 128, 512 * n:512 * n + 512],
                        ob[:])

    nc.compile()
    return nc


def _get_nc():
    if "nc" not in _cached:
        _cached["nc"] = _build()
    return _cached["nc"]


def _make_in_maps(x, Wq, bq, Wk, bk, Wv, bv, Wo):
    sc = 1.0 / np.sqrt(HD)
    tri = np.arange(128)
    mask = np.where(tri[:, None] <= tri[None, :], 0.0, MASK_VAL).astype(np.float32)
    in_maps = []
    for c in range(N_CORES):
        b, g = divmod(c, GROUPS)
        sl = slice(JG * g, JG * (g + 1))
        in_maps.append({
            "xT": np.ascontiguousarray(x[b].T).astype(BF16),
            "wqT": np.ascontiguousarray((Wq[sl] * sc).T).astype(BF16),
            "wkT": np.ascontiguousarray(Wk[sl].T).astype(BF16),
            "wvT": np.ascontiguousarray(Wv[sl].T).astype(BF16),
            "woT": np.ascontiguousarray(Wo[:, sl].T).astype(BF16),
            "bqc": (bq[sl] * sc).astype(np.float32).reshape(JG, 1),
            "bkc": bk[sl].astype(np.float32).reshape(JG, 1),
            "bvb": np.broadcast_to(bv[sl].astype(np.float32), (128, JG)).copy(),
            "maskT": mask,
        })
    return in_maps


def kernel(x, Wq, bq, Wk, bk, Wv, bv, Wo, bo, _return_results=False):
    from concourse.bass_utils import run_bass_kernel_spmd

    nc = _get_nc()
    in_maps = _make_in_maps(np.asarray(x, np.float32), np.asarray(Wq, np.float32),
                            np.asarray(bq, np.float32), np.asarray(Wk, np.float32),
                            np.asarray(bk, np.float32), np.asarray(Wv, np.float32),
                            np.asarray(bv, np.float32), np.asarray(Wo, np.float32))
    res = run_bass_kernel_spmd(nc, in_maps, core_ids=list(range(N_CORES)))
    full = np.empty((B, S, D), np.float32)
    for b in range(B):
        acc = res.results[4 * b]["out"].astype(np.float32).copy()
        for g in range(1, GROUPS):
            acc += res.results[4 * b + g]["out"]
        full[b] = acc + np.asarray(bo, np.float32)[None, :]
    if _return_results:
        return full, res
    return full


# revision 5
# speedup vs baseline: 1.0126x; 1.0126x over previous
"""Causal multi-head attention (B=2, S=2048, D=1024, 16 heads of 64) on 8 TRN2
NeuronCores.

Sharding: core c -> batch b = c//4, head-group g = c%4 (4 heads = 256 model
dims per core).  Wq/Wk/Wv column-parallel, Wo row-parallel; the 4 partial
outputs per batch are summed on the host (no collectives).

Per-core data flow (matmul compute in bf16, fp32 PSUM accumulation):
  QT = (Wq_g/8) @ x^T      [256, 2048]   (1/sqrt(hd) folded into Wq,bq)
  KT = Wk_g @ x^T          [256, 2048]
  V  = x @ Wv_g^T + bv     [2048, 256]   natural layout, ones-augmented
  attention per head pair (64-row PE tiling throughout -> no PE mode
  switches, score pairs and AV halves run concurrently in the array):
    ST[sk,sq] = K_h @ Q_h^T          two heads at row groups 0/64
    P = exp(ST + causal mask)        one ACT exp per [128,1024] (2 banks)
    AV: split sk into two K=64 halves accumulating in separate PSUM banks
        (row-tile bank rule); lhsT is ones-augmented V so row 64 = l[sq]
    preoutT = (poA+poB)[0:64] * 1/(lA+lB)
  out_partial = preoutT.T @ Wo_g^T   [2048, 1024] fp32
Host: out[b] = sum of the 4 head-group partials + bo.
"""

import numpy as np
import ml_dtypes

B, S, D = 2, 2048, 1024
HD = 64
NH = D // HD
N_CORES = 8
GROUPS = 4          # head-groups (tensor-parallel)
JG = D // GROUPS    # local dims per core = 256
NHL = JG // HD      # local heads = 4
KCH = D // 128      # contraction chunks for projections = 8
NKT = S // 128      # sk tiles = 16
NJB = S // 512      # query blocks of 512 = 4
MASK_VAL = -1e9

BF16 = ml_dtypes.bfloat16

_cached = {}


def _build():
    import concourse.bacc as bacc
    import concourse.tile as tile
    import concourse.mybir as mybir

    f32 = mybir.dt.float32
    bf16 = mybir.dt.bfloat16
    Exp = mybir.ActivationFunctionType.Exp
    add_op = mybir.AluOpType.add

    nc = bacc.Bacc("TRN2", target_bir_lowering=False, debug=False,
                   num_devices=N_CORES)

    xT = nc.dram_tensor("xT", [D, S], bf16, kind="ExternalInput").ap()
    wqT = nc.dram_tensor("wqT", [D, JG], bf16, kind="ExternalInput").ap()
    wkT = nc.dram_tensor("wkT", [D, JG], bf16, kind="ExternalInput").ap()
    wvT = nc.dram_tensor("wvT", [D, JG], bf16, kind="ExternalInput").ap()
    woT = nc.dram_tensor("woT", [JG, D], bf16, kind="ExternalInput").ap()
    bqc = nc.dram_tensor("bqc", [JG, 1], f32, kind="ExternalInput").ap()
    bkc = nc.dram_tensor("bkc", [JG, 1], f32, kind="ExternalInput").ap()
    bvb = nc.dram_tensor("bvb", [128, JG], f32, kind="ExternalInput").ap()
    maskT = nc.dram_tensor("maskT", [128, 128], f32, kind="ExternalInput").ap()
    out = nc.dram_tensor("out", [S, D], f32, kind="ExternalOutput").ap()

    with tile.TileContext(nc) as tc:
        with (
            tc.tile_pool(name="const", bufs=1) as cpool,
            tc.tile_pool(name="pbig", bufs=3) as p_pool,
            tc.tile_pool(name="small", bufs=4) as small_pool,
            tc.tile_pool(name="outp", bufs=3) as out_pool,
            tc.tile_pool(name="mm_ps", bufs=2, space="PSUM") as mm_ps,
            tc.tile_pool(name="po_ps", bufs=4, space="PSUM") as po_ps,
        ):
            # ---- constants / weights in SBUF ----
            xt_all = cpool.tile([128, KCH, S], bf16)
            nc.sync.dma_start(
                xt_all[:], xT.rearrange("(k p) s -> p k s", p=128))
            wq_sb = cpool.tile([128, KCH, JG], bf16)
            nc.sync.dma_start(
                wq_sb[:], wqT.rearrange("(k p) m -> p k m", p=128))
            wk_sb = cpool.tile([128, KCH, JG], bf16)
            nc.sync.dma_start(
                wk_sb[:], wkT.rearrange("(k p) m -> p k m", p=128))
            wv_sb = cpool.tile([128, KCH, JG], bf16)
            nc.sync.dma_start(
                wv_sb[:], wvT.rearrange("(k p) m -> p k m", p=128))
            wo_sb = cpool.tile([128, 2, D], bf16)
            nc.sync.dma_start(
                wo_sb[:], woT.rearrange("(t p) n -> p t n", p=128))
            bq_sb = cpool.tile([128, 2], f32)
            nc.sync.dma_start(bq_sb[:], bqc.rearrange("(t p) o -> p (t o)", p=128))
            bk_sb = cpool.tile([128, 2], f32)
            nc.sync.dma_start(bk_sb[:], bkc.rearrange("(t p) o -> p (t o)", p=128))
            bvb_sb = cpool.tile([128, JG], f32)
            nc.sync.dma_start(bvb_sb[:], bvb[:])
            mask_sb = cpool.tile([128, 128], f32)
            nc.sync.dma_start(mask_sb[:], maskT[:])

            qt = [cpool.tile([128, S], bf16, name=f"qt{t}") for t in range(2)]
            kt = [cpool.tile([128, S], bf16, name=f"kt{t}") for t in range(2)]
            v_all = cpool.tile([128, NKT, NHL * 65], bf16)
            nc.vector.memset(
                v_all.rearrange("p k (h c) -> p k h c", c=65)[:, :, :, 64:65], 1.0)
            po = [cpool.tile([128, S], bf16, name=f"po{t}") for t in range(2)]

            # ---- Q^T / K^T projections ----
            for w_sb, b_sb, dst in ((wq_sb, bq_sb, qt), (wk_sb, bk_sb, kt)):
                for t in range(2):
                    ps = [mm_ps.tile([128, 1024], f32, tag="mm",
                                     name=f"psproj{t}_{n}") for n in range(2)]
                    for k in range(KCH):
                        lhsT = w_sb[:, k, 128 * t:128 * t + 128]
                        for n in range(4):
                            nc.tensor.matmul(
                                ps[n // 2][:, 512 * (n % 2):512 * (n % 2) + 512],
                                lhsT=lhsT,
                                rhs=xt_all[:, k, 512 * n:512 * n + 512],
                                start=(k == 0), stop=(k == KCH - 1))
                    for n in range(2):
                        nc.vector.tensor_scalar_add(
                            dst[t][:, 1024 * n:1024 * n + 1024], ps[n][:],
                            b_sb[:, t:t + 1])

            # ---- V projection (natural layout) ----
            # one accumulation group per PSUM bank: 2 s-tiles per [128,1024]
            # tile, parked at col 0 (bank 0) and col 512 (bank 1)
            for sg in range(8):
                ps = mm_ps.tile([128, 1024], f32, tag="mm")
                for k in range(KCH):
                    for q in range(2):
                        si = 2 * sg + q
                        nc.tensor.matmul(
                            ps[:, 512 * q:512 * q + 256],
                            lhsT=xt_all[:, k, 128 * si:128 * si + 128],
                            rhs=wv_sb[:, k, :],
                            start=(k == 0), stop=(k == KCH - 1))
                for q in range(2):
                    si = 2 * sg + q
                    nc.vector.tensor_add(
                        v_all[:, si, :].rearrange(
                            "p (h c) -> p h c", c=65)[:, :, 0:64],
                        ps[:, 512 * q:512 * q + 256].rearrange(
                            "p (h c) -> p h c", c=64),
                        bvb_sb.rearrange("p (h c) -> p h c", c=64))

            # ---- attention (all 64-row mode) + output projection per block ----
            for j in range(NJB):
                nk = 4 * (j + 1)
                npair = (nk + 1) // 2
                for pair in range(2):
                    qt_t, kt_t = qt[pair], kt[pair]
                    pt = [p_pool.tile([128, NKT, 512], bf16, tag="p",
                                      name=f"pt{pair}_{j}_{hh}")
                          for hh in range(2)]
                    # scores (K=64, head pair at row groups 0/64) + exp
                    for m in range(npair):
                        for hh in range(2):
                            base = 64 * hh
                            st = mm_ps.tile([128, 1024], f32, tag="mm",
                                            name=f"st{pair}_{j}_{m}_{hh}")
                            for sub in range(2):
                                ki = 2 * m + sub
                                if ki >= nk:
                                    continue
                                d = max(0, 128 * ki - 512 * j)
                                nc.tensor.matmul(
                                    st[:, 512 * sub + d:512 * sub + 512],
                                    lhsT=kt_t[base:base + 64,
                                              128 * ki:128 * ki + 128],
                                    rhs=qt_t[base:base + 64,
                                             512 * j + d:512 * j + 512],
                                    start=True, stop=True)
                                if ki >= 4 * j:
                                    nc.vector.tensor_add(
                                        st[:, 512 * sub + d:512 * sub + d + 128],
                                        st[:, 512 * sub + d:512 * sub + d + 128],
                                        mask_sb[:])
                            nc.scalar.activation(
                                pt[hh][:, 2 * m:2 * m + 2, :], st[:], Exp)
                    # AV: two K=64 halves -> separate PSUM banks (row-tile rule)
                    pos = [po_ps.tile([65, 512], f32, tag="po",
                                      name=f"pos{pair}_{j}_{hh}_{half}")
                           for hh in range(2) for half in range(2)]
                    for ki in range(nk):
                        d = max(0, 128 * ki - 512 * j)
                        for hh in range(2):
                            h = 2 * pair + hh
                            for half in range(2):
                                pb = 64 * half
                                nc.tensor.matmul(
                                    pos[2 * hh + half][0:65, d:512],
                                    lhsT=v_all[pb:pb + 64, ki,
                                               65 * h:65 * h + 65],
                                    rhs=pt[hh][pb:pb + 64, ki, d:512],
                                    start=(ki == 0), stop=(ki == nk - 1))
                    # evac: sum halves, normalize by 1/l, write bf16
                    for hh in range(2):
                        poA, poB = pos[2 * hh], pos[2 * hh + 1]
                        bsb = small_pool.tile([65, 512], f32, tag="bsb")
                        nc.scalar.copy(bsb[:], poB[:])
                        ssum = small_pool.tile([65, 512], f32, tag="ssum")
                        nc.vector.tensor_add(ssum[:], poA[:], bsb[:])
                        recip = small_pool.tile([1, 512], f32, tag="recip")
                        nc.vector.reciprocal(recip[:], ssum[64:65, :])
                        rb = small_pool.tile([64, 512], f32, tag="rb")
                        nc.gpsimd.partition_broadcast(rb[:], recip[:])
                        nc.vector.tensor_mul(
                            po[pair][64 * hh:64 * hh + 64,
                                     512 * j:512 * j + 512],
                            ssum[0:64, :], rb[:])
                # output projection for this block's 4 query tiles
                for m in range(4 * j, 4 * j + 4):
                    ps = mm_ps.tile([128, 1024], f32, tag="mm")
                    for t in range(2):
                        for n in range(2):
                            nc.tensor.matmul(
                                ps[:, 512 * n:512 * n + 512],
                                lhsT=po[t][:, 128 * m:128 * m + 128],
                                rhs=wo_sb[:, t, 512 * n:512 * n + 512],
                                start=(t == 0), stop=(t == 1))
                    ob = out_pool.tile([128, 1024], f32, tag="ob")
                    if m % 2 == 0:
                        nc.vector.tensor_copy(ob[:], ps[:])
                    else:
                        nc.scalar.copy(ob[:], ps[:])
                    nc.sync.dma_start(out[128 * m:128 * m + 128, :], ob[:])

    nc.compile()
    return nc


def _get_nc():
    if "nc" not in _cached:
        _cached["nc"] = _build()
    return _cached["nc"]


def _make_in_maps(x, Wq, bq, Wk, bk, Wv, bv, Wo):
    sc = 1.0 / np.sqrt(HD)
    tri = np.arange(128)
    mask = np.where(tri[:, None] <= tri[None, :], 0.0, MASK_VAL).astype(np.float32)
    in_maps = []
    for c in range(N_CORES):
        b, g = divmod(c, GROUPS)
        sl = slice(JG * g, JG * (g + 1))
        in_maps.append({
            "xT": np.ascontiguousarray(x[b].T).astype(BF16),
            "wqT": np.ascontiguousarray((Wq[sl] * sc).T).astype(BF16),
            "wkT": np.ascontiguousarray(Wk[sl].T).astype(BF16),
            "wvT": np.ascontiguousarray(Wv[sl].T).astype(BF16),
            "woT": np.ascontiguousarray(Wo[:, sl].T).astype(BF16),
            "bqc": (bq[sl] * sc).astype(np.float32).reshape(JG, 1),
            "bkc": bk[sl].astype(np.float32).reshape(JG, 1),
            "bvb": np.broadcast_to(bv[sl].astype(np.float32), (128, JG)).copy(),
            "maskT": mask,
        })
    return in_maps


def kernel(x, Wq, bq, Wk, bk, Wv, bv, Wo, bo, _return_results=False):
    from concourse.bass_utils import run_bass_kernel_spmd

    nc = _get_nc()
    in_maps = _make_in_maps(np.asarray(x, np.float32), np.asarray(Wq, np.float32),
                            np.asarray(bq, np.float32), np.asarray(Wk, np.float32),
                            np.asarray(bk, np.float32), np.asarray(Wv, np.float32),
                            np.asarray(bv, np.float32), np.asarray(Wo, np.float32))
    res = run_bass_kernel_spmd(nc, in_maps, core_ids=list(range(N_CORES)))
    full = np.empty((B, S, D), np.float32)
    for b in range(B):
        acc = res.results[4 * b]["out"].astype(np.float32).copy()
        for g in range(1, GROUPS):
            acc += res.results[4 * b + g]["out"]
        full[b] = acc + np.asarray(bo, np.float32)[None, :]
    if _return_results:
        return full, res
    return full


# revision 9
# speedup vs baseline: 1.2281x; 1.2128x over previous
"""Causal multi-head attention (B=2, S=2048, D=1024, 16 heads of 64) on 8 TRN2
NeuronCores.

Sharding: core c -> batch b = c//4, head-group g = c%4 (4 heads = 256 model
dims per core).  Wq/Wk/Wv column-parallel, Wo row-parallel; the 4 partial
outputs per batch are summed on the host (no collectives).

Per-core data flow (matmul compute in bf16, fp32 PSUM accumulation):
  QT = (Wq_g/8) @ x^T      [256, 2048]   (1/sqrt(hd) folded into Wq,bq)
  KT = Wk_g @ x^T          [256, 2048]
  V  = x @ Wv_g^T + bv     [2048, 256]   natural layout, ones-augmented
  attention per head pair (64-row PE tiling throughout -> no PE mode
  switches, score pairs and AV halves run concurrently in the array):
    ST[sk,sq] = K_h @ Q_h^T          two heads at row groups 0/64
    P = exp(ST + causal mask)        one ACT exp per [128,1024] (2 banks)
    AV: split sk into two K=64 halves accumulating in separate PSUM banks
        (row-tile bank rule); lhsT is ones-augmented V so row 64 = l[sq]
    preoutT = (poA+poB)[0:64] * 1/(lA+lB)
  out_partial = preoutT.T @ Wo_g^T   [2048, 1024] fp32
Host: out[b] = sum of the 4 head-group partials + bo.
"""

import numpy as np
import ml_dtypes

B, S, D = 2, 2048, 1024
HD = 64
NH = D // HD
N_CORES = 8
GROUPS = 4          # head-groups (tensor-parallel)
JG = D // GROUPS    # local dims per core = 256
NHL = JG // HD      # local heads = 4
KCH = D // 128      # contraction chunks for projections = 8
NKT = S // 128      # sk tiles = 16
NJB = S // 512      # query blocks of 512 = 4
MASK_VAL = -1e9

BF16 = ml_dtypes.bfloat16

_cached = {}


def _build():
    import concourse.bacc as bacc
    import concourse.tile as tile
    import concourse.mybir as mybir

    f32 = mybir.dt.float32
    bf16 = mybir.dt.bfloat16
    Exp = mybir.ActivationFunctionType.Exp
    add_op = mybir.AluOpType.add

    nc = bacc.Bacc("TRN2", target_bir_lowering=False, debug=False,
                   num_devices=N_CORES)

    xT = nc.dram_tensor("xT", [D, S], bf16, kind="ExternalInput").ap()
    wqT = nc.dram_tensor("wqT", [D, JG], bf16, kind="ExternalInput").ap()
    wkT = nc.dram_tensor("wkT", [D, JG], bf16, kind="ExternalInput").ap()
    wvT = nc.dram_tensor("wvT", [D, JG], bf16, kind="ExternalInput").ap()
    woT = nc.dram_tensor("woT", [JG, D], bf16, kind="ExternalInput").ap()
    bqc = nc.dram_tensor("bqc", [JG, 1], f32, kind="ExternalInput").ap()
    bkc = nc.dram_tensor("bkc", [JG, 1], f32, kind="ExternalInput").ap()
    bvb = nc.dram_tensor("bvb", [128, JG], f32, kind="ExternalInput").ap()
    maskT = nc.dram_tensor("maskT", [128, 128], f32, kind="ExternalInput").ap()
    out = nc.dram_tensor("out", [S, D], f32, kind="ExternalOutput").ap()

    with tile.TileContext(nc) as tc:
        with (
            tc.tile_pool(name="const", bufs=1) as cpool,
            tc.tile_pool(name="pbig", bufs=3) as p_pool,
            tc.tile_pool(name="small", bufs=4) as small_pool,
            tc.tile_pool(name="outp", bufs=3) as out_pool,
            tc.tile_pool(name="mm_ps", bufs=2, space="PSUM") as mm_ps,
            tc.tile_pool(name="po_ps", bufs=4, space="PSUM") as po_ps,
        ):
            # ---- constants / weights in SBUF ----
            xt_all = cpool.tile([128, KCH, S], bf16)
            xT_r = xT.rearrange("(k p) s -> p k s", p=128)
            for k in range(KCH):
                nc.sync.dma_start(xt_all[:, k, :], xT_r[:, k, :])
            wq_sb = cpool.tile([128, KCH, JG], bf16)
            nc.sync.dma_start(
                wq_sb[:], wqT.rearrange("(k p) m -> p k m", p=128))
            wk_sb = cpool.tile([128, KCH, JG], bf16)
            nc.sync.dma_start(
                wk_sb[:], wkT.rearrange("(k p) m -> p k m", p=128))
            wv_sb = cpool.tile([128, KCH, JG], bf16)
            nc.sync.dma_start(
                wv_sb[:], wvT.rearrange("(k p) m -> p k m", p=128))
            wo_sb = cpool.tile([128, 2, D], bf16)
            nc.sync.dma_start(
                wo_sb[:], woT.rearrange("(t p) n -> p t n", p=128))
            bq_sb = cpool.tile([128, 2], f32)
            nc.sync.dma_start(bq_sb[:], bqc.rearrange("(t p) o -> p (t o)", p=128))
            bk_sb = cpool.tile([128, 2], f32)
            nc.sync.dma_start(bk_sb[:], bkc.rearrange("(t p) o -> p (t o)", p=128))
            bvb_sb = cpool.tile([128, JG], f32)
            nc.sync.dma_start(bvb_sb[:], bvb[:])
            mask_sb = cpool.tile([128, 128], f32)
            nc.sync.dma_start(mask_sb[:], maskT[:])

            qt = [cpool.tile([128, S], bf16, name=f"qt{t}") for t in range(2)]
            kt = [cpool.tile([128, S], bf16, name=f"kt{t}") for t in range(2)]
            v_all = cpool.tile([128, NKT, NHL * 65], bf16)
            nc.vector.memset(
                v_all.rearrange("p k (h c) -> p k h c", c=65)[:, :, :, 64:65], 1.0)
            po = [cpool.tile([128, S], bf16, name=f"po{t}") for t in range(2)]

            # ---- Q^T / K^T projections ----
            for w_sb, b_sb, dst in ((wq_sb, bq_sb, qt), (wk_sb, bk_sb, kt)):
                for t in range(2):
                    ps = [mm_ps.tile([128, 1024], f32, tag="mm",
                                     name=f"psproj{t}_{n}") for n in range(2)]
                    for k in range(KCH):
                        lhsT = w_sb[:, k, 128 * t:128 * t + 128]
                        for n in range(4):
                            nc.tensor.matmul(
                                ps[n // 2][:, 512 * (n % 2):512 * (n % 2) + 512],
                                lhsT=lhsT,
                                rhs=xt_all[:, k, 512 * n:512 * n + 512],
                                start=(k == 0), stop=(k == KCH - 1))
                    for n in range(2):
                        nc.vector.tensor_scalar_add(
                            dst[t][:, 1024 * n:1024 * n + 1024], ps[n][:],
                            b_sb[:, t:t + 1])

            # ---- V projection (natural layout) ----
            # one accumulation group per PSUM bank: 2 s-tiles per [128,1024]
            # tile, parked at col 0 (bank 0) and col 512 (bank 1)
            for sg in range(8):
                ps = mm_ps.tile([128, 1024], f32, tag="mm")
                for k in range(KCH):
                    for q in range(2):
                        si = 2 * sg + q
                        nc.tensor.matmul(
                            ps[:, 512 * q:512 * q + 256],
                            lhsT=xt_all[:, k, 128 * si:128 * si + 128],
                            rhs=wv_sb[:, k, :],
                            start=(k == 0), stop=(k == KCH - 1))
                for q in range(2):
                    si = 2 * sg + q
                    nc.vector.tensor_add(
                        v_all[:, si, :].rearrange(
                            "p (h c) -> p h c", c=65)[:, :, 0:64],
                        ps[:, 512 * q:512 * q + 256].rearrange(
                            "p (h c) -> p h c", c=64),
                        bvb_sb.rearrange("p (h c) -> p h c", c=64))

            # ---- attention (all 64-row mode) + output projection per block ----
            for j in range(NJB):
                nk = 4 * (j + 1)
                npair = (nk + 1) // 2
                for pair in range(2):
                    qt_t, kt_t = qt[pair], kt[pair]
                    pt = [p_pool.tile([128, NKT, 512], bf16, tag="p",
                                      name=f"pt{pair}_{j}_{hh}")
                          for hh in range(2)]
                    # scores (K=64, head pair at row groups 0/64) + exp
                    for m in range(npair):
                        for hh in range(2):
                            base = 64 * hh
                            st = mm_ps.tile([128, 1024], f32, tag="mm",
                                            name=f"st{pair}_{j}_{m}_{hh}")
                            for sub in range(2):
                                ki = 2 * m + sub
                                if ki >= nk:
                                    continue
                                d = max(0, 128 * ki - 512 * j)
                                nc.tensor.matmul(
                                    st[:, 512 * sub + d:512 * sub + 512],
                                    lhsT=kt_t[base:base + 64,
                                              128 * ki:128 * ki + 128],
                                    rhs=qt_t[base:base + 64,
                                             512 * j + d:512 * j + 512],
                                    start=True, stop=True)
                                if ki >= 4 * j:
                                    nc.vector.tensor_add(
                                        st[:, 512 * sub + d:512 * sub + d + 128],
                                        st[:, 512 * sub + d:512 * sub + d + 128],
                                        mask_sb[:])
                            nc.scalar.activation(
                                pt[hh][:, 2 * m:2 * m + 2, :], st[:], Exp)
                    # AV: two K=64 halves -> separate PSUM banks (row-tile rule)
                    pos = [po_ps.tile([65, 512], f32, tag="po",
                                      name=f"pos{pair}_{j}_{hh}_{half}")
                           for hh in range(2) for half in range(2)]
                    for ki in range(nk):
                        d = max(0, 128 * ki - 512 * j)
                        for hh in range(2):
                            h = 2 * pair + hh
                            for half in range(2):
                                pb = 64 * half
                                nc.tensor.matmul(
                                    pos[2 * hh + half][0:65, d:512],
                                    lhsT=v_all[pb:pb + 64, ki,
                                               65 * h:65 * h + 65],
                                    rhs=pt[hh][pb:pb + 64, ki, d:512],
                                    start=(ki == 0), stop=(ki == nk - 1))
                    # evac: sum halves, normalize by 1/l, write bf16
                    for hh in range(2):
                        poA, poB = pos[2 * hh], pos[2 * hh + 1]
                        bsb = small_pool.tile([65, 512], f32, tag="bsb")
                        nc.vector.tensor_copy(bsb[:], poB[:])
                        ssum = small_pool.tile([65, 512], f32, tag="ssum")
                        nc.vector.tensor_add(ssum[:], poA[:], bsb[:])
                        lrow = small_pool.tile([1, 512], f32, tag="lrow")
                        nc.vector.tensor_copy(lrow[:], ssum[64:65, :])
                        rb = small_pool.tile([64, 512], f32, tag="rb")
                        nc.gpsimd.partition_broadcast(rb[:], lrow[:])
                        rbr = small_pool.tile([64, 512], f32, tag="rbr")
                        nc.vector.reciprocal_approx_fast(rbr[:], rb[:])
                        nc.vector.tensor_mul(
                            po[pair][64 * hh:64 * hh + 64,
                                     512 * j:512 * j + 512],
                            ssum[0:64, :], rbr[:])
                # output projection, deferred one block for scheduling slack
                wo_js = [j - 1] if j >= 1 else []
                if j == NJB - 1:
                    wo_js.append(j)
                for wj in wo_js:
                    for m in range(4 * wj, 4 * wj + 4):
                        ps = mm_ps.tile([128, 1024], f32, tag="mm")
                        for t in range(2):
                            for n in range(2):
                                nc.tensor.matmul(
                                    ps[:, 512 * n:512 * n + 512],
                                    lhsT=po[t][:, 128 * m:128 * m + 128],
                                    rhs=wo_sb[:, t, 512 * n:512 * n + 512],
                                    start=(t == 0), stop=(t == 1))
                        ob = out_pool.tile([128, 1024], f32, tag="ob")
                        if m % 2 == 0:
                            nc.vector.tensor_copy(ob[:], ps[:])
                        else:
                            nc.scalar.copy(ob[:], ps[:])
                        nc.sync.dma_start(out[128 * m:128 * m + 128, :], ob[:])

    nc.compile()
    return nc


def _get_nc():
    if "nc" not in _cached:
        _cached["nc"] = _build()
    return _cached["nc"]


def _make_in_maps(x, Wq, bq, Wk, bk, Wv, bv, Wo):
    sc = 1.0 / np.sqrt(HD)
    tri = np.arange(128)
    mask = np.where(tri[:, None] <= tri[None, :], 0.0, MASK_VAL).astype(np.float32)
    in_maps = []
    for c in range(N_CORES):
        b, g = divmod(c, GROUPS)
        sl = slice(JG * g, JG * (g + 1))
        in_maps.append({
            "xT": np.ascontiguousarray(x[b].T).astype(BF16),
            "wqT": np.ascontiguousarray((Wq[sl] * sc).T).astype(BF16),
            "wkT": np.ascontiguousarray(Wk[sl].T).astype(BF16),
            "wvT": np.ascontiguousarray(Wv[sl].T).astype(BF16),
            "woT": np.ascontiguousarray(Wo[:, sl].T).astype(BF16),
            "bqc": (bq[sl] * sc).astype(np.float32).reshape(JG, 1),
            "bkc": bk[sl].astype(np.float32).reshape(JG, 1),
            "bvb": np.broadcast_to(bv[sl].astype(np.float32), (128, JG)).copy(),
            "maskT": mask,
        })
    return in_maps


def kernel(x, Wq, bq, Wk, bk, Wv, bv, Wo, bo, _return_results=False):
    from concourse.bass_utils import run_bass_kernel_spmd

    nc = _get_nc()
    in_maps = _make_in_maps(np.asarray(x, np.float32), np.asarray(Wq, np.float32),
                            np.asarray(bq, np.float32), np.asarray(Wk, np.float32),
                            np.asarray(bk, np.float32), np.asarray(Wv, np.float32),
                            np.asarray(bv, np.float32), np.asarray(Wo, np.float32))
    res = run_bass_kernel_spmd(nc, in_maps, core_ids=list(range(N_CORES)))
    full = np.empty((B, S, D), np.float32)
    for b in range(B):
        acc = res.results[4 * b]["out"].astype(np.float32).copy()
        for g in range(1, GROUPS):
            acc += res.results[4 * b + g]["out"]
        full[b] = acc + np.asarray(bo, np.float32)[None, :]
    if _return_results:
        return full, res
    return full
